# revision 1
# baseline (speedup 1.0000x reference)
"""Instruction-minimal Trainium2 Bass kernel for nn_GCN_15590731285230 (v2.2).

On this rig every engine instruction costs ~25-45us of dispatch (globally
serialized), so the objective is MINIMUM TOTAL INSTRUCTION COUNT, not
engine-seconds. Design:

  * Scores only in transposed layout ST[m, n] = S[n, m] via fp32 matmuls
    (fp32 matmuls self-load weights: no separate Ldweights instruction).
  * Pass A fills one 8-bank PSUM tensor [128, 4096] (two m-chunks) per group,
    halving the PSUM->SBUF drain copies.
  * Softmax over m fused across the whole score tensor:
      - max: one strided DVE reduce over chunks + one gpsimd
        partition_all_reduce (arrives broadcast to all 128 partitions)
      - one tensor_sub over all 16 chunks using a stride-0 broadcast AP
      - one in-place Exp over the whole [128, 16*2048] tensor
      - sums: one strided DVE chunk-sum + one partition_all_reduce
      - normalization folded into Z: scale ZT columns by 1/s (exact: the same
        exp values feed the sums and the Z matmuls).
  * Layer-2 shortcut: only row 0 of layer 2 is needed;
    v = sum_n h1[n,:] R[0,n] via tensor_mul + reduce against a
    partition-broadcast of R's row 0 (extracted straight from the exp'd
    scores, column n=0).
  * SBUF slot reuse via pool tags: yT -> znorm -> wsum, mtile -> stile ->
    r0tile, mx_pt -> r0row (stall alone is 128KB/partition).

Per batch b (core b):
    R  = softmax(x wr x^T, axis=-1);  h1 = relu(R x w1 + x)
    out_b = relu(R[0,:] @ h1 @ w2 + h1[0,:])
"""

import sys

if "/opt/trn_rl_repo" not in sys.path:
    sys.path.insert(0, "/opt/trn_rl_repo")

from contextlib import ExitStack

import numpy as np

import concourse.bacc as bacc
import concourse.bass as bass
import concourse.bass_isa as bass_isa
import concourse.mybir as mybir
import concourse.tile as tile
from concourse.bass_utils import run_bass_kernel_spmd
from concourse.masks import make_identity

P = 128
D = 128
B = 8
F32 = mybir.dt.float32
AF = mybir.ActivationFunctionType
AX = mybir.AxisListType
ALU = mybir.AluOpType
ROP = bass_isa.ReduceOp


def _bcast_free(ap, count):
    """Insert a stride-0 dim of size `count` after the partition dim."""
    return bass.AP(tensor=ap.tensor, offset=ap.offset,
                   ap=[list(ap.ap[0]), [0, count]] + [list(d) for d in ap.ap[1:]])


def build_kernel(n=2048, repeat=1):
    nt = n // P              # m chunks
    w5 = min(512, n)         # matmul moving-operand width
    pair = 2 if nt % 2 == 0 else 1   # chunks per PSUM drain in pass A
    nc = bacc.Bacc()
    x_d = nc.dram_tensor("x", [n, D], F32, kind="ExternalInput")
    wr_d = nc.dram_tensor("wr", [D, D], F32, kind="ExternalInput")
    w1_d = nc.dram_tensor("w1", [D, D], F32, kind="ExternalInput")
    w2_d = nc.dram_tensor("w2", [D, D], F32, kind="ExternalInput")
    out_d = nc.dram_tensor("out", [1, D], F32, kind="ExternalOutput")

    with tile.TileContext(nc) as tc, ExitStack() as ctx:
        sg = ctx.enter_context(tc.tile_pool(name="sg", bufs=1))
        scr = ctx.enter_context(tc.tile_pool(name="scr", bufs=1))
        bb = ctx.enter_context(tc.tile_pool(name="bb", bufs=1))
        st = ctx.enter_context(tc.tile_pool(name="st", bufs=1))

        for _rep in range(repeat):
            ident = sg.tile([P, P], F32, tag="ident")
            make_identity(nc, ident)
            wr_sb = sg.tile([P, P], F32, tag="wr_sb")
            w1_sb = sg.tile([P, P], F32, tag="w1_sb")
            w2_sb = sg.tile([P, P], F32, tag="w2_sb")
            nc.sync.dma_start(wr_sb, wr_d[:])
            nc.sync.dma_start(w1_sb, w1_d[:])
            nc.sync.dma_start(w2_sb, w2_d[:])

            # One DMA: partition p holds x rows p*nt..p*nt+nt-1 (contiguous 8KB
            # per partition). This permutes the node enumeration to
            # g(p,t) = p*nt + t, which is consistent everywhere downstream
            # (scores, Z, h1 permute simultaneously; position 0 is still node
            # 0, and the output depends only on node 0's row).
            xnat = sg.tile([P, nt, P], F32, tag="xnat")
            nc.sync.dma_start(xnat, x_d[:].rearrange("(p t) f -> p t f", p=P))

            # xT via PE transposes packed into one wide PSUM tensor; then yT
            # matmuls into a second one. One drain copy per half instead of
            # one per 4-chunk group.
            xT = sg.tile([P, n], F32, tag="xT")
            yT = sg.tile([P, n], F32, tag="yT")
            with tc.tile_pool(name="pst", bufs=2, space="PSUM") as pst:
                half = max(n // 2, P)
                for h in range(0, n, half):
                    tp = pst.tile([P, half], F32, tag="tp")
                    for k in range(half // P):
                        nc.tensor.transpose(tp[:, k * P:(k + 1) * P],
                                            xnat[:, h // P + k, :], ident)
                    nc.vector.tensor_copy(xT[:, h:h + half], tp)
                # yT = (x @ wr)^T : yT[g, n] = sum_f wr[f, g] xT[f, n]
                wy = min(w5, half)
                for h in range(0, n, half):
                    yp = pst.tile([P, half], F32, tag="tp")
                    for j in range(0, half, wy):
                        nc.tensor.matmul(yp[:, j:j + wy], lhsT=wr_sb,
                                         rhs=xT[:, h + j:h + j + wy],
                                         start=True, stop=True)
                    nc.vector.tensor_copy(yT[:, h:h + half], yp)

            # ---- pass A: ST[m, n] = S[n, m], stored fp32 in SBUF ----
            stall = sg.tile([P, nt, n], F32, tag="stall")
            stall_flat = stall.rearrange("p t n -> p (t n)")
            with tc.tile_pool(name="psA", bufs=1, space="PSUM") as psA:
                for g in range(0, nt, pair):
                    sp = psA.tile([P, pair * n], F32, tag="sp")
                    for k in range(pair):
                        for j in range(0, n, w5):
                            nc.tensor.matmul(
                                sp[:, k * n + j:k * n + j + w5],
                                lhsT=xT[:, (g + k) * P:(g + k + 1) * P],
                                rhs=yT[:, j:j + w5],
                                start=True, stop=True)
                    nc.scalar.copy(
                        stall_flat[:, g * n:(g + pair) * n], sp)

            # ---- global column max over m (partitions x chunks) ----
            mx_pt = scr.tile([P, n], F32, tag="scr")
            nc.vector.tensor_reduce(mx_pt, stall.rearrange("p t n -> p n t"),
                                    axis=AX.X, op=ALU.max)
            mtile = bb.tile([P, n], F32, tag="bb")
            nc.gpsimd.partition_all_reduce(mtile, mx_pt, channels=P,
                                           reduce_op=ROP.max)

            # ---- softmax numerator: one sub + one in-place exp ----
            nc.vector.tensor_sub(stall_flat, stall_flat,
                                 _bcast_free(mtile[:], nt))
            nc.scalar.activation(stall_flat, stall_flat, AF.Exp)

            # column sums s[n] (over chunks, then partitions)
            etsum = scr.tile([P, n], F32, tag="scr")
            nc.vector.tensor_reduce(etsum, stall.rearrange("p t n -> p n t"),
                                    axis=AX.X, op=ALU.add)
            stile = bb.tile([P, n], F32, tag="bb")   # reuses mtile slot
            nc.gpsimd.partition_all_reduce(stile, etsum, channels=P,
                                           reduce_op=ROP.add)
            nc.vector.reciprocal(stile, stile)       # 1/s, broadcast

            # ---- Z^T accumulation over chunks (fp32, self-loading mms) ----
            with tc.tile_pool(name="psB", bufs=1, space="PSUM") as psB:
                ztp = psB.tile([P, n], F32, tag="zt")
                for t in range(nt):
                    for j in range(0, n, w5):
                        nc.tensor.matmul(ztp[:, j:j + w5],
                                         lhsT=xnat[:, t, :],
                                         rhs=stall[:, t, j:j + w5],
                                         start=(t == 0), stop=(t == nt - 1))
                # znorm = ZT * (1/s): one op does PSUM->SBUF copy and scale
                znorm = sg.tile([P, n], F32, tag="yT")   # reuses yT slot
                nc.vector.tensor_mul(znorm, ztp, stile)

                # ---- h1T = relu(w1^T Znorm + xT) ----
                h1t = sg.tile([P, n], F32, tag="h1t")
                hp = psB.tile([P, n], F32, tag="hp")
                for j in range(0, n, w5):
                    nc.tensor.matmul(hp[:, j:j + w5], lhsT=w1_sb,
                                     rhs=znorm[:, j:j + w5],
                                     start=True, stop=True)
                nc.vector.tensor_add(h1t, hp, xT)
                nc.vector.tensor_relu(h1t, h1t)

                # ---- tail: out = relu(r0 @ h1 @ w2 + h1[0, :]) ----
                # r0 (unnormalized) = exp'd scores column n=0 = stall[:, :, 0]
                rtp = psB.tile([nt, P], F32, tag="zt")  # reuses ztp banks
                nc.tensor.transpose(
                    rtp, stall[:, :, 0:1].rearrange("p t o -> p (t o)"),
                    ident)
                r16 = st.tile([nt, P], F32, tag="r16")
                nc.vector.tensor_copy(r16, rtp)
                r0row = scr.tile([1, n], F32, tag="scr")
                nc.sync.dma_start(
                    r0row.rearrange("o (t p) -> o t p", t=nt), r16)
                # normalize by 1/s[0] (stile holds reciprocals, broadcast)
                nc.vector.tensor_scalar_mul(r0row, r0row, stile[0:1, 0:1])
                r0tile = bb.tile([P, n], F32, tag="bb")  # reuses stile slot
                nc.gpsimd.partition_broadcast(r0tile, r0row)
                wsum = sg.tile([P, n], F32, tag="yT")    # reuses znorm slot
                nc.vector.tensor_mul(wsum, h1t, r0tile)
                v = st.tile([P, 1], F32, tag="v")
                nc.vector.tensor_reduce(v, wsum, axis=AX.X, op=ALU.add)
                o2 = psB.tile([1, P], F32, tag="hp")  # reuses hp banks
                nc.tensor.matmul(o2, lhsT=v, rhs=w2_sb, start=True, stop=False)
                nc.tensor.matmul(o2, lhsT=h1t[:, 0:1], rhs=ident,
                                 start=False, stop=True)
                fin = st.tile([1, P], F32, tag="fin")
                nc.scalar.activation(fin, o2, AF.Relu)
                nc.sync.dma_start(out_d[:], fin)

    nc.compile()
    return nc


_CACHE = {}


def kernel(x, w1, w2, wr):
    x = np.ascontiguousarray(np.asarray(x), dtype=np.float32)
    w1 = np.ascontiguousarray(np.asarray(w1), dtype=np.float32)
    w2 = np.ascontiguousarray(np.asarray(w2), dtype=np.float32)
    wr = np.ascontiguousarray(np.asarray(wr), dtype=np.float32)
    b, n, d = x.shape
    if "nc" not in _CACHE:
        _CACHE["nc"] = build_kernel(n)
    nc = _CACHE["nc"]
    in_maps = [{"x": x[i], "wr": wr, "w1": w1, "w2": w2} for i in range(b)]
    res = run_bass_kernel_spmd(nc, in_maps, core_ids=list(range(b)))
    return np.stack([res.results[i]["out"][0] for i in range(b)])



# revision 5
# speedup vs baseline: 2.0019x; 2.0019x over previous
"""Trainium2 Bass kernel for nn_GCN_15590731285230 (v3.0).

Rig cost profile (microbenched 2026-08-09): gpsimd ops (partition_all_reduce /
partition_broadcast) cost ~0.7ms EACH; matmuls ~11us flat; big DVE/ACT passes
~us per 256 elems/partition. v3 removes ALL gpsimd work:

  * partition-direction max/sum of the score stats are done by 16 PE
    transposes of the [128, 2048] chunk-reduced stats into PSUM followed by
    ONE strided DVE reduce -> [128, 16] per-column stats.
  * stats are reassembled to a [1, 2048] row (PE transpose + small DMA,
    same pattern as the baseline's r0 extraction) and broadcast to all 128
    partitions with ones-vector matmuls (contraction dim 1) into PSUM.
  * all matmuls stay fp32: fp32 matmuls self-load weights (one instruction);
    16-bit matmuls would emit a separate Ldweights per call.

Per batch b (core b):
    R  = softmax(x wr x^T, axis=-1);  h1 = relu(R x w1 + x)
    out_b = relu(R[0,:] @ h1 @ w2 + h1[0,:])
"""

import sys

if "/opt/trn_rl_repo" not in sys.path:
    sys.path.insert(0, "/opt/trn_rl_repo")

from contextlib import ExitStack

import numpy as np

import concourse.bacc as bacc
import concourse.bass as bass
import concourse.mybir as mybir
import concourse.tile as tile
from concourse.bass_utils import run_bass_kernel_spmd
from concourse.masks import make_identity

P = 128
D = 128
B = 8
F32 = mybir.dt.float32
AF = mybir.ActivationFunctionType
AX = mybir.AxisListType
ALU = mybir.AluOpType


def _bcast_free(ap, count):
    """Insert a stride-0 dim of size `count` after the partition dim."""
    return bass.AP(tensor=ap.tensor, offset=ap.offset,
                   ap=[list(ap.ap[0]), [0, count]] + [list(d) for d in ap.ap[1:]])


def build_kernel(n=2048, repeat=1):
    nt = n // P              # m chunks
    w5 = min(512, n)         # matmul moving-operand width
    pair = 2 if nt % 2 == 0 else 1   # chunks per PSUM drain in pass A
    nc = bacc.Bacc()
    x_d = nc.dram_tensor("x", [n, D], F32, kind="ExternalInput")
    wr_d = nc.dram_tensor("wr", [D, D], F32, kind="ExternalInput")
    w1_d = nc.dram_tensor("w1", [D, D], F32, kind="ExternalInput")
    w2_d = nc.dram_tensor("w2", [D, D], F32, kind="ExternalInput")
    out_d = nc.dram_tensor("out", [1, D], F32, kind="ExternalOutput")

    with tile.TileContext(nc) as tc, ExitStack() as ctx:
        sg = ctx.enter_context(tc.tile_pool(name="sg", bufs=1))
        scr = ctx.enter_context(tc.tile_pool(name="scr", bufs=1))
        st = ctx.enter_context(tc.tile_pool(name="st", bufs=1))

        for _rep in range(repeat):
            ident = sg.tile([P, P], F32, tag="ident")
            make_identity(nc, ident)
            ones1 = sg.tile([1, P], F32, tag="ones1")
            nc.vector.memset(ones1, 1.0)
            wr_sb = sg.tile([P, P], F32, tag="wr_sb")
            w1_sb = sg.tile([P, P], F32, tag="w1_sb")
            w2_sb = sg.tile([P, P], F32, tag="w2_sb")
            nc.sync.dma_start(wr_sb, wr_d[:])
            nc.sync.dma_start(w1_sb, w1_d[:])
            nc.sync.dma_start(w2_sb, w2_d[:])

            # One DMA: partition p holds x rows p*nt..p*nt+nt-1. Node
            # enumeration everywhere downstream is c = k*128 + p <-> row
            # p*nt + k; position 0 is still node 0.
            xnat = sg.tile([P, nt, P], F32, tag="xnat")
            nc.sync.dma_start(xnat, x_d[:].rearrange("(p t) f -> p t f", p=P))

            # xT via PE transposes packed into wide PSUM tensors; yT = (x wr)^T.
            xT = sg.tile([P, n], F32, tag="xT")
            yT = sg.tile([P, n], F32, tag="yT")
            with tc.tile_pool(name="pst", bufs=2, space="PSUM") as pst:
                half = max(n // 2, P)
                for h in range(0, n, half):
                    tp = pst.tile([P, half], F32, tag="tp")
                    for k in range(half // P):
                        nc.tensor.transpose(tp[:, k * P:(k + 1) * P],
                                            xnat[:, h // P + k, :], ident)
                    nc.vector.tensor_copy(xT[:, h:h + half], tp)
                wy = min(w5, half)
                for h in range(0, n, half):
                    yp = pst.tile([P, half], F32, tag="tp")
                    for j in range(0, half, wy):
                        nc.tensor.matmul(yp[:, j:j + wy], lhsT=wr_sb,
                                         rhs=xT[:, h + j:h + j + wy],
                                         start=True, stop=True)
                    nc.vector.tensor_copy(yT[:, h:h + half], yp)

            # ---- pass A: ST[m, n] = S[n, m], fp32 in SBUF ----
            stall = sg.tile([P, nt, n], F32, tag="stall")
            stall_flat = stall.rearrange("p t n -> p (t n)")
            with tc.tile_pool(name="psA", bufs=1, space="PSUM") as psA:
                for g in range(0, nt, pair):
                    sp = psA.tile([P, pair * n], F32, tag="sp")
                    for k in range(pair):
                        for j in range(0, n, w5):
                            nc.tensor.matmul(
                                sp[:, k * n + j:k * n + j + w5],
                                lhsT=xT[:, (g + k) * P:(g + k + 1) * P],
                                rhs=yT[:, j:j + w5],
                                start=True, stop=True)
                    nc.scalar.copy(
                        stall_flat[:, g * n:(g + pair) * n], sp)

            # ---- softmax stats, gpsimd-free ----
            # Chunk-reduced per-partition stats [128, n], then partition
            # direction handled by PE transposes + one strided DVE reduce.
            mx_pt = scr.tile([P, n], F32, tag="scr")
            nc.vector.tensor_reduce(mx_pt, stall.rearrange("p t n -> p n t"),
                                    axis=AX.X, op=ALU.max)
            with tc.tile_pool(name="psS", bufs=1, space="PSUM") as psS:
                # column max: transpose 128-blocks of mx_pt, reduce over m
                mxT = psS.tile([P, n], F32, tag="xt")
                for j in range(nt):
                    nc.tensor.transpose(mxT[:, j * P:(j + 1) * P],
                                        mx_pt[:, j * P:(j + 1) * P], ident)
                colmax = st.tile([P, nt], F32, tag="colmax")
                nc.vector.tensor_reduce(
                    colmax, mxT.rearrange("p (j m) -> p j m", j=nt),
                    axis=AX.X, op=ALU.max)
                # reassemble to a [1, n] row: value for column c=j*128+p
                cmT = psS.tile([nt, P], F32, tag="xt")  # mxT slot is free now
                nc.tensor.transpose(cmT, colmax, ident)
                cm16 = st.tile([nt, P], F32, tag="cm16")
                nc.vector.tensor_copy(cm16, cmT)
                mrow = scr.tile([1, n], F32, tag="mrow")
                nc.sync.dma_start(
                    mrow.rearrange("o (j p) -> o j p", j=nt), cm16)
                # broadcast to all 128 partitions via ones-matmuls into PSUM
                mtile = psS.tile([P, n], F32, tag="bc")
                for j in range(0, n, w5):
                    nc.tensor.matmul(mtile[:, j:j + w5], lhsT=ones1,
                                     rhs=mrow[:, j:j + w5],
                                     start=True, stop=True)

                # ---- softmax numerator: one sub + one in-place exp ----
                nc.vector.tensor_sub(stall_flat, stall_flat,
                                     _bcast_free(mtile[:], nt))
            nc.scalar.activation(stall_flat, stall_flat, AF.Exp)

            # column sums, same structure
            et_pt = scr.tile([P, n], F32, tag="scr")
            nc.vector.tensor_reduce(et_pt, stall.rearrange("p t n -> p n t"),
                                    axis=AX.X, op=ALU.add)
            stile_sb = sg.tile([P, n], F32, tag="stile_sb")
            rrow = scr.tile([1, n], F32, tag="mrow")
            with tc.tile_pool(name="psS2", bufs=1, space="PSUM") as psS2:
                etT = psS2.tile([P, n], F32, tag="xt")
                for j in range(nt):
                    nc.tensor.transpose(etT[:, j * P:(j + 1) * P],
                                        et_pt[:, j * P:(j + 1) * P], ident)
                colsum = st.tile([P, nt], F32, tag="colmax")
                nc.vector.tensor_reduce(
                    colsum, etT.rearrange("p (j m) -> p j m", j=nt),
                    axis=AX.X, op=ALU.add)
                csT = psS2.tile([nt, P], F32, tag="xt")  # etT slot is free now
                nc.tensor.transpose(csT, colsum, ident)
                cs16 = st.tile([nt, P], F32, tag="cm16")
                nc.vector.tensor_copy(cs16, csT)
                srow = scr.tile([1, n], F32, tag="srow")
                nc.sync.dma_start(
                    srow.rearrange("o (j p) -> o j p", j=nt), cs16)
                nc.vector.reciprocal(rrow, srow)    # 1/s as a row
                stile = psS2.tile([P, n], F32, tag="bc")
                for j in range(0, n, w5):
                    nc.tensor.matmul(stile[:, j:j + w5], lhsT=ones1,
                                     rhs=rrow[:, j:j + w5],
                                     start=True, stop=True)
                nc.scalar.copy(stile_sb, stile)     # keep past pool close

            # ---- Z^T accumulation over chunks (fp32, self-loading mms) ----
            with tc.tile_pool(name="psB", bufs=1, space="PSUM") as psB:
                ztp = psB.tile([P, n], F32, tag="zt")
                for t in range(nt):
                    for j in range(0, n, w5):
                        nc.tensor.matmul(ztp[:, j:j + w5],
                                         lhsT=xnat[:, t, :],
                                         rhs=stall[:, t, j:j + w5],
                                         start=(t == 0), stop=(t == nt - 1))
                # znorm = ZT * (1/s): PSUM->SBUF copy and scale in one op
                znorm = sg.tile([P, n], F32, tag="yT")   # reuses yT slot
                nc.vector.tensor_mul(znorm, ztp, stile_sb)

                # ---- h1T = relu(w1^T Znorm + xT) ----
                h1t = sg.tile([P, n], F32, tag="h1t")
                hp = psB.tile([P, n], F32, tag="hp")
                for j in range(0, n, w5):
                    nc.tensor.matmul(hp[:, j:j + w5], lhsT=w1_sb,
                                     rhs=znorm[:, j:j + w5],
                                     start=True, stop=True)
                nc.vector.tensor_add(h1t, hp, xT)
                nc.vector.tensor_relu(h1t, h1t)

                # ---- tail: out = relu(r0 @ h1 @ w2 + h1[0, :]) ----
                # r0 (unnormalized) = exp'd scores column n=0 = stall[:, :, 0]
                rtp = psB.tile([nt, P], F32, tag="zt")  # reuses ztp banks
                nc.tensor.transpose(
                    rtp, stall[:, :, 0:1].rearrange("p t o -> p (t o)"),
                    ident)
                r16 = st.tile([nt, P], F32, tag="r16")
                nc.vector.tensor_copy(r16, rtp)
                r0row = scr.tile([1, n], F32, tag="srow")
                nc.sync.dma_start(
                    r0row.rearrange("o (t p) -> o t p", t=nt), r16)
                # normalize by 1/s[0] (rrow holds reciprocals)
                nc.vector.tensor_scalar_mul(r0row, r0row, rrow[0:1, 0:1])
                # broadcast r0 to all partitions via ones-matmuls
                r0tile = psB.tile([P, n], F32, tag="hp")  # reuses hp banks
                for j in range(0, n, w5):
                    nc.tensor.matmul(r0tile[:, j:j + w5], lhsT=ones1,
                                     rhs=r0row[:, j:j + w5],
                                     start=True, stop=True)
                wsum = sg.tile([P, n], F32, tag="yT")    # reuses znorm slot
                nc.vector.tensor_mul(wsum, h1t, r0tile)
                v = st.tile([P, 1], F32, tag="v")
                nc.vector.tensor_reduce(v, wsum, axis=AX.X, op=ALU.add)
                o2 = psB.tile([1, P], F32, tag="zt")  # rtp/ztp banks are free
                nc.tensor.matmul(o2, lhsT=v, rhs=w2_sb, start=True, stop=False)
                nc.tensor.matmul(o2, lhsT=h1t[:, 0:1], rhs=ident,
                                 start=False, stop=True)
                fin = st.tile([1, P], F32, tag="fin")
                nc.scalar.activation(fin, o2, AF.Relu)
                nc.sync.dma_start(out_d[:], fin)

    nc.compile()
    return nc


_CACHE = {}


def kernel(x, w1, w2, wr):
    x = np.ascontiguousarray(np.asarray(x), dtype=np.float32)
    w1 = np.ascontiguousarray(np.asarray(w1), dtype=np.float32)
    w2 = np.ascontiguousarray(np.asarray(w2), dtype=np.float32)
    wr = np.ascontiguousarray(np.asarray(wr), dtype=np.float32)
    b, n, d = x.shape
    if "nc" not in _CACHE:
        _CACHE["nc"] = build_kernel(n)
    nc = _CACHE["nc"]
    in_maps = [{"x": x[i], "wr": wr, "w1": w1, "w2": w2} for i in range(b)]
    res = run_bass_kernel_spmd(nc, in_maps, core_ids=list(range(b)))
    return np.stack([res.results[i]["out"][0] for i in range(b)])


# revision 8
# speedup vs baseline: 2.2534x; 1.1256x over previous
"""Trainium2 Bass kernel for nn_GCN_15590731285230 (v3.0).

Rig cost profile (microbenched 2026-08-09): gpsimd ops (partition_all_reduce /
partition_broadcast) cost ~0.7ms EACH; matmuls ~11us flat; big DVE/ACT passes
~us per 256 elems/partition. v3 removes ALL gpsimd work:

  * partition-direction max/sum of the score stats are done by 16 PE
    transposes of the [128, 2048] chunk-reduced stats into PSUM followed by
    ONE strided DVE reduce -> [128, 16] per-column stats.
  * stats are reassembled to a [1, 2048] row (PE transpose + small DMA,
    same pattern as the baseline's r0 extraction) and broadcast to all 128
    partitions with ones-vector matmuls (contraction dim 1) into PSUM.
  * all matmuls stay fp32: fp32 matmuls self-load weights (one instruction);
    16-bit matmuls would emit a separate Ldweights per call.

Per batch b (core b):
    R  = softmax(x wr x^T, axis=-1);  h1 = relu(R x w1 + x)
    out_b = relu(R[0,:] @ h1 @ w2 + h1[0,:])
"""

import sys

if "/opt/trn_rl_repo" not in sys.path:
    sys.path.insert(0, "/opt/trn_rl_repo")

from contextlib import ExitStack

import numpy as np

import concourse.bacc as bacc
import concourse.bass as bass
import concourse.mybir as mybir
import concourse.tile as tile
from concourse.bass_utils import run_bass_kernel_spmd
from concourse.masks import make_identity

P = 128
D = 128
B = 8
F32 = mybir.dt.float32
AF = mybir.ActivationFunctionType
AX = mybir.AxisListType
ALU = mybir.AluOpType


def _bcast_free(ap, count):
    """Insert a stride-0 dim of size `count` after the partition dim."""
    return bass.AP(tensor=ap.tensor, offset=ap.offset,
                   ap=[list(ap.ap[0]), [0, count]] + [list(d) for d in ap.ap[1:]])


def build_kernel(n=2048, repeat=1):
    nt = n // P              # m chunks
    w5 = min(512, n)         # matmul moving-operand width
    pair = 2 if nt % 2 == 0 else 1   # chunks per PSUM drain in pass A
    nc = bacc.Bacc()
    x_d = nc.dram_tensor("x", [n, D], F32, kind="ExternalInput")
    wp_d = nc.dram_tensor("wpack", [3, D, D], F32, kind="ExternalInput")
    out_d = nc.dram_tensor("out", [1, D], F32, kind="ExternalOutput")

    with tile.TileContext(nc) as tc, ExitStack() as ctx:
        sg = ctx.enter_context(tc.tile_pool(name="sg", bufs=1))
        scr = ctx.enter_context(tc.tile_pool(name="scr", bufs=1))
        st = ctx.enter_context(tc.tile_pool(name="st", bufs=1))

        for _rep in range(repeat):
            ident = sg.tile([P, P], F32, tag="ident")
            make_identity(nc, ident)
            ones1 = sg.tile([1, P], F32, tag="ones1")
            nc.vector.memset(ones1, 1.0)
            wp_sb = sg.tile([P, 3, P], F32, tag="wp_sb")
            nc.sync.dma_start(wp_sb, wp_d[:].rearrange("w p f -> p w f"))
            wr_sb = wp_sb[:, 0, :]
            w1_sb = wp_sb[:, 1, :]
            w2_sb = wp_sb[:, 2, :]

            # One DMA: partition p holds x rows p*nt..p*nt+nt-1. Node
            # enumeration everywhere downstream is c = k*128 + p <-> row
            # p*nt + k; position 0 is still node 0.
            xnat = sg.tile([P, nt, P], F32, tag="xnat")
            nc.sync.dma_start(xnat, x_d[:].rearrange("(p t) f -> p t f", p=P))

            # xT via PE transposes packed into wide PSUM tensors; yT = (x wr)^T.
            xT = sg.tile([P, n], F32, tag="xT")
            yT = sg.tile([P, n], F32, tag="yT")
            with tc.tile_pool(name="pst", bufs=2, space="PSUM") as pst:
                half = max(n // 2, P)
                for h in range(0, n, half):
                    tp = pst.tile([P, half], F32, tag="tp")
                    for k in range(half // P):
                        nc.tensor.transpose(tp[:, k * P:(k + 1) * P],
                                            xnat[:, h // P + k, :], ident)
                    nc.vector.tensor_copy(xT[:, h:h + half], tp)
                wy = min(w5, half)
                for h in range(0, n, half):
                    yp = pst.tile([P, half], F32, tag="tp")
                    for j in range(0, half, wy):
                        nc.tensor.matmul(yp[:, j:j + wy], lhsT=wr_sb,
                                         rhs=xT[:, h + j:h + j + wy],
                                         start=True, stop=True)
                    nc.vector.tensor_copy(yT[:, h:h + half], yp)

            # ---- pass A: ST[m, n] = S[n, m], fp32 in SBUF ----
            stall = sg.tile([P, nt, n], F32, tag="stall")
            stall_flat = stall.rearrange("p t n -> p (t n)")
            with tc.tile_pool(name="psA", bufs=1, space="PSUM") as psA:
                for g in range(0, nt, pair):
                    sp = psA.tile([P, pair * n], F32, tag="sp")
                    for k in range(pair):
                        for j in range(0, n, w5):
                            nc.tensor.matmul(
                                sp[:, k * n + j:k * n + j + w5],
                                lhsT=xT[:, (g + k) * P:(g + k + 1) * P],
                                rhs=yT[:, j:j + w5],
                                start=True, stop=True)
                    nc.scalar.copy(
                        stall_flat[:, g * n:(g + pair) * n], sp)

            # ---- softmax stats, gpsimd-free ----
            # Chunk-reduced per-partition stats [128, n], then partition
            # direction handled by PE transposes + one strided DVE reduce.
            mx_pt = scr.tile([P, n], F32, tag="scr")
            nc.vector.tensor_reduce(mx_pt, stall.rearrange("p t n -> p n t"),
                                    axis=AX.X, op=ALU.max)
            with tc.tile_pool(name="psS", bufs=1, space="PSUM") as psS:
                # column max: transpose 128-blocks of mx_pt, reduce over m
                mxT = psS.tile([P, n], F32, tag="xt")
                for j in range(nt):
                    nc.tensor.transpose(mxT[:, j * P:(j + 1) * P],
                                        mx_pt[:, j * P:(j + 1) * P], ident)
                colmax = st.tile([P, nt], F32, tag="colmax")
                nc.vector.tensor_reduce(
                    colmax, mxT.rearrange("p (j m) -> p j m", j=nt),
                    axis=AX.X, op=ALU.max)
                # reassemble to a [1, n] row: value for column c=j*128+p
                cmT = psS.tile([nt, P], F32, tag="xt")  # mxT slot is free now
                nc.tensor.transpose(cmT, colmax, ident)
                cm16 = st.tile([nt, P], F32, tag="cm16")
                nc.vector.tensor_copy(cm16, cmT)
                mrow = scr.tile([1, n], F32, tag="mrow")
                nc.sync.dma_start(
                    mrow.rearrange("o (j p) -> o j p", j=nt), cm16)
                # broadcast to all 128 partitions via ones-matmuls into PSUM
                mtile = psS.tile([P, n], F32, tag="bc")
                for j in range(0, n, w5):
                    nc.tensor.matmul(mtile[:, j:j + w5], lhsT=ones1,
                                     rhs=mrow[:, j:j + w5],
                                     start=True, stop=True)

                # ---- softmax numerator: one sub + one in-place exp ----
                nc.vector.tensor_sub(stall_flat, stall_flat,
                                     _bcast_free(mtile[:], nt))
            nc.scalar.activation(stall_flat, stall_flat, AF.Exp)

            # column sums, same structure
            et_pt = scr.tile([P, n], F32, tag="scr")
            nc.vector.tensor_reduce(et_pt, stall.rearrange("p t n -> p n t"),
                                    axis=AX.X, op=ALU.add)
            stile_sb = sg.tile([P, n], F32, tag="stile_sb")
            rrow = scr.tile([1, n], F32, tag="mrow")
            with tc.tile_pool(name="psS2", bufs=1, space="PSUM") as psS2:
                etT = psS2.tile([P, n], F32, tag="xt")
                for j in range(nt):
                    nc.tensor.transpose(etT[:, j * P:(j + 1) * P],
                                        et_pt[:, j * P:(j + 1) * P], ident)
                colsum = st.tile([P, nt], F32, tag="colmax")
                nc.vector.tensor_reduce(
                    colsum, etT.rearrange("p (j m) -> p j m", j=nt),
                    axis=AX.X, op=ALU.add)
                csT = psS2.tile([nt, P], F32, tag="xt")  # etT slot is free now
                nc.tensor.transpose(csT, colsum, ident)
                cs16 = st.tile([nt, P], F32, tag="cm16")
                nc.vector.tensor_copy(cs16, csT)
                srow = scr.tile([1, n], F32, tag="srow")
                nc.sync.dma_start(
                    srow.rearrange("o (j p) -> o j p", j=nt), cs16)
                nc.vector.reciprocal(rrow, srow)    # 1/s as a row
                stile = psS2.tile([P, n], F32, tag="bc")
                for j in range(0, n, w5):
                    nc.tensor.matmul(stile[:, j:j + w5], lhsT=ones1,
                                     rhs=rrow[:, j:j + w5],
                                     start=True, stop=True)
                nc.scalar.copy(stile_sb, stile)     # keep past pool close

            # ---- Z^T accumulation over chunks (fp32, self-loading mms) ----
            with tc.tile_pool(name="psB", bufs=1, space="PSUM") as psB:
                ztp = psB.tile([P, n], F32, tag="zt")
                for t in range(nt):
                    for j in range(0, n, w5):
                        nc.tensor.matmul(ztp[:, j:j + w5],
                                         lhsT=xnat[:, t, :],
                                         rhs=stall[:, t, j:j + w5],
                                         start=(t == 0), stop=(t == nt - 1))
                # znorm = ZT * (1/s): PSUM->SBUF copy and scale in one op
                znorm = sg.tile([P, n], F32, tag="yT")   # reuses yT slot
                nc.vector.tensor_mul(znorm, ztp, stile_sb)

                # ---- h1T = relu(w1^T Znorm + xT) ----
                h1t = sg.tile([P, n], F32, tag="h1t")
                hp = psB.tile([P, n], F32, tag="hp")
                for j in range(0, n, w5):
                    nc.tensor.matmul(hp[:, j:j + w5], lhsT=w1_sb,
                                     rhs=znorm[:, j:j + w5],
                                     start=True, stop=True)
                nc.vector.tensor_add(h1t, hp, xT)
                nc.vector.tensor_relu(h1t, h1t)

                # ---- tail: out = relu(r0 @ h1 @ w2 + h1[0, :]) ----
                # r0 (unnormalized) = exp'd scores column n=0 = stall[:, :, 0]
                rtp = psB.tile([nt, P], F32, tag="zt")  # reuses ztp banks
                nc.tensor.transpose(
                    rtp, stall[:, :, 0:1].rearrange("p t o -> p (t o)"),
                    ident)
                r16 = st.tile([nt, P], F32, tag="r16")
                nc.vector.tensor_copy(r16, rtp)
                r0row = scr.tile([1, n], F32, tag="srow")
                nc.sync.dma_start(
                    r0row.rearrange("o (t p) -> o t p", t=nt), r16)
                # normalize by 1/s[0] (rrow holds reciprocals)
                nc.vector.tensor_scalar_mul(r0row, r0row, rrow[0:1, 0:1])
                # broadcast r0 to all partitions via ones-matmuls
                r0tile = psB.tile([P, n], F32, tag="hp")  # reuses hp banks
                for j in range(0, n, w5):
                    nc.tensor.matmul(r0tile[:, j:j + w5], lhsT=ones1,
                                     rhs=r0row[:, j:j + w5],
                                     start=True, stop=True)
                wsum = sg.tile([P, n], F32, tag="yT")    # reuses znorm slot
                nc.vector.tensor_mul(wsum, h1t, r0tile)
                v = st.tile([P, 1], F32, tag="v")
                nc.vector.tensor_reduce(v, wsum, axis=AX.X, op=ALU.add)
                o2 = psB.tile([1, P], F32, tag="zt")  # rtp/ztp banks are free
                nc.tensor.matmul(o2, lhsT=v, rhs=w2_sb, start=True, stop=False)
                nc.tensor.matmul(o2, lhsT=h1t[:, 0:1], rhs=ident,
                                 start=False, stop=True)
                fin = st.tile([1, P], F32, tag="fin")
                nc.scalar.activation(fin, o2, AF.Relu)
                nc.sync.dma_start(out_d[:], fin)

    nc.compile()
    return nc


_CACHE = {}


def kernel(x, w1, w2, wr):
    x = np.ascontiguousarray(np.asarray(x), dtype=np.float32)
    w1 = np.ascontiguousarray(np.asarray(w1), dtype=np.float32)
    w2 = np.ascontiguousarray(np.asarray(w2), dtype=np.float32)
    wr = np.ascontiguousarray(np.asarray(wr), dtype=np.float32)
    b, n, d = x.shape
    if "nc" not in _CACHE:
        _CACHE["nc"] = build_kernel(n)
    nc = _CACHE["nc"]
    wpack = np.ascontiguousarray(np.stack([wr, w1, w2]))
    in_maps = [{"x": x[i], "wpack": wpack} for i in range(b)]
    res = run_bass_kernel_spmd(nc, in_maps, core_ids=list(range(b)))
    return np.stack([res.results[i]["out"][0] for i in range(b)])
